# revision 26
# baseline (speedup 1.0000x reference)
"""Trainium2 Bass kernel for the CAM-drop attention module.

Reference computation (per sample n):
    cams  = relu(W @ x[n])            # W: [C=64, Cin=1024], x[n]: [Cin, H*W]
    thr_k = gama * max_hw(cams[k])    # per-channel spatial max
    drop  = where(cams > thr, 0, cams)
    out[n] = x[n] * mean_k(drop)      # broadcast over Cin

Data-parallel over the batch: 32 samples sharded 4-per-core across 8
NeuronCores; fc_weights / gama replicated. No cross-core communication.

The problem is HBM-bound, so x is pre-cast to bf16 on the host and loaded
as bf16, and the output is stored as bf16 and widened to f32 on the host
(halves both HBM streams; rel err stays ~7e-3, well under the 2e-2 gate).
Matmuls accumulate bf16 into f32 PSUM; the channel mean is bf16.

Per-core pipeline (samples unrolled):
  - ALL x loads AND output stores share the single sync HWDGE ring, with
    store emission deferred until after all load issuance: the ring is
    FIFO, so the full 25.7 MB load stream drains at ~420 GB/s before any
    store bytes move -- a structural priority no scheduler reordering can
    break -- and SWDGE is avoided entirely (its Q7 descriptor rings
    contend with SDMA engines 7/15's AXI ports and grew a ~13us
    single-engine tail on the last load tile). Total time only depends on
    total bytes while the ring stays busy, so the lost load/store overlap
    is free. Consts (w prelaid [128, 512] on host, gama) ride the
    otherwise-idle scalar ring.
  - x[n] streamed as 8 bf16 tiles [128, 3136] into a 30-slot rotating
    SBUF pool; the 2 slot-recycled loads (30-31) sit right after the two
    sample-0 stores that free their slots.
  - cams in f32 PSUM, one tile per bank: sample 0 t-outer over banks 0-6
    (matmuls chase the initial tile loads); samples 1+ chunk-pair serial
    over banks 0-3 ONLY, with the mean matmuls on banks 4-6, so sample
    n+1's cam matmuls never wait on sample n's mean work (that bank
    coupling cost 10-14us of DVE idle per sample in the t-outer layout).
  - per-bank relu (ACT) -> two partial spatial maxes + final (DVE),
    threshold, in-place drop-mask (DVE scalar_tensor_tensor)
  - channel mean via a bf16 [64->128] ones/64 matmul into banks 4-6,
    copied per-bank PSUM->SBUF on ACT
  - products IN PLACE: xb tile *= mean (DVE 2x tensor_tensor); tile 0
    chunked per bank to chase the mean copies; per-tile stores
  - host widens the bf16 output back to f32

Measured pitfalls baked into the structure: GpSimd tensor ops contend
with DVE tensor_tensor for the shared SBUF read port (both ~4x slower);
ScalarE ACTIVATE has no 16-bit accel; HAM power throttling (50%-duty
windows ~half the time) stretches PE matmuls ~1.65x; Tile lowers
cross-engine deps to per-engine completion COUNTERS, so an op can
transitively wait on unrelated earlier ops the scheduler placed ahead of
its producer (keep each sample's serial chain tight on few engines);
in-order engine queues mean a hoisted load-paced matmul stream can block
a later-emitted mean matmul (keep the last sample's loads early).

End state: purely DMA-bound. The DVE stream (dense, ~23.6us/sample, no
inter-sample gaps) finishes at ~124us while the ring stays busy from
~8.5us to ~137us: ramp (~8.5us framework sem setup) + 51.4 MB at ~420
GB/s + a load->store transition dip + completion. Known-negative
variants (measured): fp8 anywhere (rel-err gate), 128-partition packed
stats via PE tile_position col-offset (works numerically but denser
activity trips HAM 50%-duty ~80% of the run), mean copies on DVE
(adds more serial DVE than the ACT-queue coupling it removes), loads
split across both HWDGE rings (ACT can't issue DMAs timely), stores
gated behind a dummy-read SWDGE transfer (scheduler hoists past it).

Run-to-run variance (~140-160us for THIS binary) is dominated by the
transition dip, not DVE/HAM: on hot runs the dip grows ~3.5 -> ~13us
(ring idle 70-85us) because loads 30-31 wait their WAR stores plus a
variable per-engine (E7/E15) load tail, and the bulk stores sit BEHIND
them in ring FIFO order so nothing fills the window. UNTESTED designed
fix for a future session: emit st(0,2..7) (~4.7MB, ready by ~70us)
between the two WAR stores and ld(3,6)/ld(3,7) -- the ring then drains
useful store bytes during the WAR wait, while tiles 30-31 still land
~84us, well before sample 3's chain consumes them (~102us). Do NOT
move st(1,*) there too: that would push tiles 30-31 past ~100us into
the sample-3 critical path.
"""

import numpy as np

# Problem shape (hardcoded per harness contract).
N, CIN, H, W = 32, 1024, 56, 56
C = 64
HW = H * W          # 3136
NCORES = 8
NS = N // NCORES    # 4 samples per core
P = 128             # SBUF partitions
NT = CIN // P       # 8 Cin tiles
NCH = 7             # spatial chunks per sample
CH = HW // NCH      # 448 (fits one PSUM bank)
BANK = 512          # PSUM bank stride in f32 elements
NBBUF = 30          # rotating bf16 x-tile slots (0.784 MB each)

_CACHE = {}


def _build_nc():
    from concourse import bacc, bass, tile
    from concourse import mybir

    f32 = mybir.dt.float32
    bf16 = mybir.dt.bfloat16
    alu = mybir.AluOpType

    nc = bacc.Bacc("TRN2", target_bir_lowering=False, debug=False)
    x_ext = nc.declare_dram_parameter("x", [NS, CIN, HW], bf16, isOutput=False)
    # fc_weights prelaid on host as [p, t*C+c] = w[c, t*128+p]: contiguous
    # 1KB partition lines -> one efficient DMA (the [CIN, C] layout's 128B
    # lines ran at ~24 GB/s and stalled the load ring for ~10us at startup).
    w_ext = nc.declare_dram_parameter("fc_weights", [P, NT * C], bf16, isOutput=False)
    g_ext = nc.declare_dram_parameter("gama", [C, 2], f32, isOutput=False)
    out_ext = nc.declare_dram_parameter("out", [NS, CIN, HW], bf16, isOutput=True)

    with tile.TileContext(nc) as tc:
        with (
            tc.tile_pool(name="consts", bufs=1) as constp,
            tc.tile_pool(name="xbp", bufs=NBBUF) as xbp,
            tc.tile_pool(name="stats", bufs=2) as statp,
            tc.tile_pool(name="camsb", bufs=1) as camp,
            tc.tile_pool(name="meanp", bufs=1) as meanp,
            tc.tile_pool(name="psum", bufs=1, space=bass.MemorySpace.PSUM) as psump,
        ):
            all_xbs = []
            deferred = []
            # Consts go on the scalar HWDGE ring so the sync ring starts
            # streaming x immediately (loads and consts in parallel).
            w_sb = constp.tile([P, NT, C], bf16)
            nc.scalar.dma_start(
                out=w_sb[:].rearrange("p a b -> p (a b)"), in_=w_ext[:, :]
            )
            # Columns: (gama, -gama).
            g_sb = constp.tile([C, 2], f32)
            nc.scalar.dma_start(out=g_sb[:], in_=g_ext[:])
            ones_sb = constp.tile([C, P], bf16)
            nc.vector.memset(ones_sb[:], 1.0 / C)

            # PE clock warm-up: the HAM gate holds the PE at half clock until
            # ~4us of sustained matmul activity. Garbage matmuls into a spare
            # PSUM bank (never read; DCE keeps unread matmuls) warm it up
            # during the initial load-only DMA phase.
            warm_ps = psump.tile([C, BANK], f32, name="warm_ps", tag="warm")
            w_flat = w_sb[:].rearrange("p a b -> p (a b)")
            for _ in range(15):
                nc.tensor.matmul(
                    warm_ps[:, :], w_sb[:, 0, :], w_flat[:, 0:BANK],
                    start=True, stop=True,
                )

            for n in range(NS):
                # ALL loads and stores share the single sync HWDGE ring.
                # The ring drains FIFO, so the 30 load DMAs queued first
                # complete before any store bytes move -- a structural
                # "stores after loads" gate no scheduler reordering can
                # break, with zero cost to total time (only total bytes
                # matter while the ring stays busy). It also eliminates
                # SWDGE entirely: the Q7 store-descriptor rings contend
                # with SDMA engines 7/15's AXI ports, which grew a ~13us
                # single-engine tail on the last load tile in the v10
                # trace. Store EMISSION is deferred so the sync engine's
                # per-store product-waits sit after all load issuance.
                xbs = []
                for t in range(NT):
                    xb = xbp.tile([P, HW], bf16, name=f"xb_{n}_{t}", tag="xb")
                    nc.sync.dma_start(out=xb[:], in_=x_ext[n, t * P:(t + 1) * P, :])
                    xbs.append(xb)
                    if n == NS - 1 and t == 5:
                        # Loads 30-31 recycle the SBUF slots of sample 0
                        # tiles 0-1, so those two stores must complete
                        # first: emit them between ld(3,5) and ld(3,6),
                        # followed by four more ready sample-0 stores
                        # (~3.1MB) so the ring drains useful bytes during
                        # the WAR wait instead of idling -- on hot runs
                        # that window grew to ~13us (ring idle 70-85us)
                        # as the variable SDMA-7/15 load tail stretched
                        # the chain. Only four fillers: tiles 30-31 must
                        # still land by ~85us since ALL of sample 3's cam
                        # matmul groups consume tile 31 (t-inner order).
                        for dn, dt in deferred[:6]:
                            nc.sync.dma_start(
                                out=out_ext[dn, dt * P:(dt + 1) * P, :],
                                in_=all_xbs[dn][dt][:],
                            )
                all_xbs.append(xbs)

                if n == NS - 1:
                    for dn, dt in deferred[6:]:
                        nc.sync.dma_start(
                            out=out_ext[dn, dt * P:(dt + 1) * P, :],
                            in_=all_xbs[dn][dt][:],
                        )

                # PSUM budget: cams(n>=1) cycle banks 0-3 only (chunk-pair
                # serial below), mean uses banks 4-6, so sample n+1's cam
                # matmuls never wait on sample n's mean copies -- the v3
                # trace showed that bank coupling as 10-14us DVE idle gaps
                # before samples 2 and 3.
                cams = [
                    psump.tile([P, BANK], f32, name=f"cams_{n}_{s}",
                               tag=f"bank{s if n == 0 else s % 4}")
                    for s in range(NCH)
                ]
                crelu = camp.tile([C, NCH, CH], bf16, name=f"crelu_{n}", tag="crelu")
                if n == 0:
                    # Sample 0: t-outer (7 live banks) so matmuls chase the
                    # initial tile loads; banks are all free at startup.
                    for t in range(NT):
                        for s in range(NCH):
                            nc.tensor.matmul(
                                cams[s][0:C, 0:CH],
                                w_sb[:, t, :],
                                xbs[t][:, s * CH:(s + 1) * CH],
                                start=(t == 0),
                                stop=(t == NT - 1),
                            )
                    for s in range(NCH):
                        nc.scalar.activation(
                            crelu[:, s, :], cams[s][0:C, 0:CH],
                            mybir.ActivationFunctionType.Relu,
                        )
                else:
                    # Samples 1+: chunk-pair serial -- only 2 banks live per
                    # group, cycling banks 0-3; per-group relus evacuate
                    # banks two groups ahead of reuse. Loads are prefetched
                    # ~3.5 samples ahead so the t-inner order never stalls.
                    for chunks in ((0, 1), (2, 3), (4, 5), (6,)):
                        for t in range(NT):
                            for s in chunks:
                                nc.tensor.matmul(
                                    cams[s][0:C, 0:CH],
                                    w_sb[:, t, :],
                                    xbs[t][:, s * CH:(s + 1) * CH],
                                    start=(t == 0),
                                    stop=(t == NT - 1),
                                )
                        for s in chunks:
                            nc.scalar.activation(
                                crelu[:, s, :], cams[s][0:C, 0:CH],
                                mybir.ActivationFunctionType.Relu,
                            )
                # Spatial max in two partials chasing the relus; final max
                # combines. max(crelu) == relu(max(cams)), so thr =
                # max(crelu) * gama directly (and -thr via the -gama col).
                cmax2 = statp.tile([C, 2], f32, name=f"cmax2_{n}", tag="cmax2")
                nc.vector.tensor_reduce(
                    cmax2[:, 0:1], crelu[:, 0:4, :], axis=mybir.AxisListType.XY,
                    op=alu.max,
                )
                nc.vector.tensor_reduce(
                    cmax2[:, 1:2], crelu[:, 4:NCH, :], axis=mybir.AxisListType.XY,
                    op=alu.max,
                )
                cmax = statp.tile([C, 1], f32, name=f"cmax_{n}", tag="cmax")
                nc.vector.tensor_reduce(
                    cmax[:], cmax2[:], axis=mybir.AxisListType.X, op=alu.max
                )
                thr = statp.tile([C, 1], f32, name=f"thr_{n}", tag="thr")
                nc.vector.tensor_scalar(
                    thr[:], cmax[:], g_sb[:, 0:1], None, op0=alu.mult
                )

                # drop = crelu * (crelu <= thr), in place (comparing post-relu
                # values against thr >= 0 matches the reference's pre-relu
                # compare). Then the channel mean, broadcast to all 128
                # partitions via a ones/64 matmul into banks 4-6 (disjoint
                # from the cams cycle on banks 0-3).
                mean_ps = [
                    psump.tile([P, BANK], f32, name=f"meanps_{n}_{s}",
                               tag=f"bank{4 + s % 3}")
                    for s in range(NCH)
                ]
                mean_sb = meanp.tile([P, HW], bf16, name=f"mean_{n}", tag="mean")
                mean_sb3 = mean_sb[:].rearrange("p (a b) -> p a b", a=NCH)
                # Mask as the fused scalar_tensor_tensor (1x mode but a
                # single pass): splitting it into a 4x is_le + 2x multiply
                # measured WORSE end-to-end -- the denser 4x op activity
                # trips HAM power throttling (50% duty windows).
                for s0, s1 in ((0, 4), (4, NCH)):
                    nc.vector.scalar_tensor_tensor(
                        crelu[:, s0:s1, :], crelu[:, s0:s1, :], thr[:],
                        crelu[:, s0:s1, :], op0=alu.is_le, op1=alu.mult,
                    )
                for s in range(NCH):
                    nc.tensor.matmul(
                        mean_ps[s][:, 0:CH], ones_sb[:], crelu[:, s, :],
                        start=True, stop=True,
                    )
                for s in range(NCH):
                    nc.scalar.copy(mean_sb3[:, s, :], mean_ps[s][:, 0:CH])

                # Products overwrite the xb tiles in place (no separate out
                # pool -> 6 more xb slots of load prefetch). Tile 0 is
                # chunked per bank so it chases the ACT copies, and its
                # store goes out on the (idle) scalar HWDGE ring. All
                # products stay on DVE: a GpSimd tensor op running
                # concurrently with DVE 2x-mode ops contends for SBUF ports
                # and slows BOTH ~4x (measured 1.78us -> 7.7us).
                xb0 = xbs[0][:].rearrange("p (a b) -> p a b", a=NCH)
                for s in range(NCH):
                    nc.vector.tensor_mul(
                        xb0[:, s, :], xb0[:, s, :], mean_sb3[:, s, :]
                    )
                # Stores: the last sample's issue inline (the ring is past
                # the loads by then); samples 0-2 are deferred to the
                # iteration-3 emission block above so their product-waits
                # never stall load issuance on the sync engine.
                if n == NS - 1:
                    nc.sync.dma_start(out=out_ext[n, 0:P, :], in_=xbs[0][:])
                else:
                    deferred.append((n, 0))
                for t in range(1, NT):
                    nc.vector.tensor_mul(xbs[t][:], xbs[t][:], mean_sb[:])
                    if n == NS - 1:
                        nc.sync.dma_start(
                            out=out_ext[n, t * P:(t + 1) * P, :], in_=xbs[t][:]
                        )
                    else:
                        deferred.append((n, t))
    nc.compile()
    return nc


def _get_nc():
    if "nc" not in _CACHE:
        _CACHE["nc"] = _build_nc()
    return _CACHE["nc"]


def _make_in_maps(x, fc_weights, gama):
    from concourse import mybir

    bf16_np = mybir.dt.np(mybir.dt.bfloat16)
    x = np.asarray(x, dtype=np.float32)
    # [p, t*C+c] = w[c, t*128+p]: one contiguous [128, 512] block.
    w2 = np.ascontiguousarray(
        np.asarray(fc_weights, dtype=np.float32)
        .reshape(C, NT, P)
        .transpose(2, 1, 0)
        .reshape(P, NT * C)
    ).astype(bf16_np)
    g = np.asarray(gama, dtype=np.float32).reshape(1, 1)
    g64 = np.ascontiguousarray(
        np.broadcast_to(np.concatenate([g, -g], axis=1), (C, 2))
    )
    return [
        {
            "x": np.ascontiguousarray(
                x[i * NS:(i + 1) * NS].reshape(NS, CIN, HW)
            ).astype(bf16_np),
            "fc_weights": w2,
            "gama": g64,
        }
        for i in range(NCORES)
    ]


def kernel(x: np.ndarray, fc_weights: np.ndarray, gama: np.ndarray) -> np.ndarray:
    from concourse.bass_utils import run_bass_kernel_spmd

    nc = _get_nc()
    in_maps = _make_in_maps(x, fc_weights, gama)
    res = run_bass_kernel_spmd(nc, in_maps, core_ids=list(range(NCORES)))
    out = np.concatenate(
        [
            res.results[i]["out"].astype(np.float32).reshape(NS, CIN, H, W)
            for i in range(NCORES)
        ],
        axis=0,
    )
    return out

